# revision 1
# baseline (speedup 1.0000x reference)
"""Multi-head attention forward on 8 Trainium2 NeuronCores.

Problem: batch=8, seq=1024, d_model=1024, n_heads=16, d_head=64, fp32 ref.

Sharding: data-parallel over batch - core b computes batch element b end to
end (weights replicated, no collectives).

Per-core layout strategy (nothing ever needs an on-device transpose):
  - x^T (d on partitions) is staged by the host; it serves as
      rhs  for Q^T/K^T = W^T @ x^T   (2 heads packed -> M=128)
      lhsT for V      = x @ W_V      (heads along the free dim)
  - scores^T = K @ Q^T lands with k on partitions, so softmax's exp is one
    ScalarE activation per tile (the 1/sqrt(d) scale and the key-mask fold
    in as activation scale/bias), and the sum over k happens inside the
    P@V matmul via a ones-column appended to V (softmax denominators pop
    out in psum row 64 for free).
  - Z^T = [V|1]^T @ P^T keeps (head, e) on partitions; heads are packed in
    pairs so the output projection contracts with K=128.
  - biases are exact rank-1 (K=1) matmul updates; they are emitted FIRST in
    each accumulation group so the group's psum-reuse (WAR) wait rides on
    them.

Everything is bf16 into the PE with fp32 PSUM accumulation.

This toolchain's walrus encodes at most ONE sync wait per instruction, so
the program is structured to need at most one new foreign-engine tick per
instruction: every load DMA writes a fresh (never reused) tile so it needs
zero waits, a few tiny observer matmuls at the top absorb the DMA-lane
ticks of shared tensors, and phase-internal pipelines are arranged so each
instruction picks up at most one new semaphore.
"""

from contextlib import ExitStack

import numpy as np

import concourse.bass as bass
import concourse.tile as tile
from concourse import mybir
from concourse.bass_utils import run_bass_kernel_spmd

S = 1024  # seq
D = 1024  # d_model
H = 16  # heads
E = 64  # d_head
B = 8  # batch == n_cores
P = 128  # partitions
NS = S // P  # 8 s-tiles
ND = D // P  # 8 d-chunks
NG = H // 2  # 8 head pairs

F32 = mybir.dt.float32
BF16 = mybir.dt.bfloat16
AF = mybir.ActivationFunctionType

MASK_NEG = 60.0  # exp(x - 60) ~ 9e-27: masked keys vanish without inf/nan


def build_program(split_waits=True):
    nc = bass.Bass("TRN2", target_bir_lowering=False, debug=False)

    # all inputs arrive pre-packed by the host into their exact SBUF layouts
    xt_d = nc.dram_tensor("xt", [P, ND, S], BF16, kind="ExternalInput").ap()
    wq_d = nc.dram_tensor("wq", [P, NG, ND, P], BF16, kind="ExternalInput").ap()
    wk_d = nc.dram_tensor("wk", [P, NG, ND, P], BF16, kind="ExternalInput").ap()
    wv_d = nc.dram_tensor("wv", [P, ND, H * E], BF16, kind="ExternalInput").ap()
    wo_d = nc.dram_tensor("wo", [P, NG, D], BF16, kind="ExternalInput").ap()
    # packed small constants: [b_Q | b_K | b_V | b_O], each flattened to 1024
    cpk_d = nc.dram_tensor("cpk", [1, 4 * 1024], BF16, kind="ExternalInput").ap()
    mb_d = nc.dram_tensor("mb", [P, NS], F32, kind="ExternalInput").ap()
    out_d = nc.dram_tensor("out", [S, D], F32, kind="ExternalOutput").ap()

    with tile.TileContext(nc) as tc, ExitStack() as ctx:
        g1 = ctx.enter_context(tc.tile_pool(name="g1", bufs=1))

        ones_row = g1.tile([1, S], BF16, tag="ones_row")
        nc.vector.memset(ones_row, 1.0)
        ones_col = g1.tile([1, P], BF16, tag="ones_col")
        nc.vector.memset(ones_col, 1.0)
        mb_sb = g1.tile([P, NS], F32, tag="mb")
        nc.sync.dma_start(out=mb_sb, in_=mb_d)
        cpk = g1.tile([1, 4 * 1024], BF16, tag="cpk")
        nc.sync.dma_start(out=cpk, in_=cpk_d)
        bq_sb = cpk[:, 0 : H * E]
        bk_sb = cpk[:, H * E : 2 * H * E]
        bv_sb = cpk[:, 2 * H * E : 3 * H * E]
        bo_sb = cpk[:, 3 * H * E : 4 * H * E]

        # xT: [d%128, d-chunk, s] - one DMA, one semaphore lane
        xT = g1.tile([P, ND, S], BF16, tag="xT")
        nc.sync.dma_start(out=xT, in_=xt_d)
        # wo: [(h%2)*64+e, pair g, d] - one DMA
        wo_sb = g1.tile([P, NG, D], BF16, tag="wo_sb")
        nc.sync.dma_start(out=wo_sb, in_=wo_d)

        # persistent activations
        qT = g1.tile([P, NG, S], BF16, tag="qT")
        kT = g1.tile([P, NG, S], BF16, tag="kT")
        vb = g1.tile([P, NS, H, E + 1], BF16, tag="vb")
        nc.vector.memset(vb, 1.0)  # pre-sets the softmax-sum ones columns
        zT = g1.tile([P, NG, S], BF16, tag="zT")

        # observer ldweights: absorb one new semaphore tick each on PE, so
        # later consumers of these tensors carry at most one wait themselves.
        nc.tensor.ldweights(ones_col)  # DVE tick (memsets)
        nc.tensor.ldweights(cpk[:, 0:P])  # cpk DMA lane
        nc.tensor.ldweights(xT[:, 0, 0:8])  # xT DMA lane
        nc.tensor.ldweights(wo_sb[:, 0, 0:8])  # wo DMA lane
        act_scrap = g1.tile([P, 1], F32, tag="act_scrap")
        nc.scalar.activation(  # mb DMA lane, observed by ScalarE
            out=act_scrap, in_=mb_sb[:, 0:1], func=AF.Copy
        )

        _projections(nc, tc, xT, wq_d, wk_d, wv_d, qT, kT, vb,
                     bq_sb, bk_sb, bv_sb, ones_row, ones_col)
        _attention(nc, tc, qT, kT, vb, zT, mb_sb, ones_row, ones_col)
        _out_proj(nc, tc, zT, wo_sb, bo_sb, ones_col, out_d)

    if split_waits:
        _split_multi_waits(nc)
    return nc


def _split_multi_waits(nc):
    """This walrus build encodes at most ONE sync wait per instruction.
    Tile emits more. Hoist excess waits onto same-engine EventSemaphore
    instructions inserted immediately before the offender - engines and
    DGE sequencers execute their streams in order, so this preserves
    semantics exactly."""
    n = 0
    for fn in nc.m.functions:
        for bb in fn.blocks:
            out = []
            for inst in bb.instructions:
                si = getattr(inst, "sync_info", None)
                waits = list(si.on_wait) if si is not None and si.on_wait else []
                if len(waits) > 1:
                    for w in waits[:-1]:
                        n += 1
                        out.append(
                            mybir.InstEventSemaphore(
                                name=f"evw-{n}",
                                engine=inst.engine,
                                sync_info=mybir.SyncInfo(
                                    on_wait=[w], on_update=[]
                                ),
                            )
                        )
                    si.on_wait = [waits[-1]]
                out.append(inst)
            bb.instructions[:] = out


def _projections(nc, tc, xT, wq_d, wk_d, wv_d, qT, kT, vb,
                 bq_sb, bk_sb, bv_sb, ones_row, ones_col):
    with (
        tc.tile_pool(name="wqk", bufs=1) as wqkp,
        tc.tile_pool(name="wvp", bufs=1) as wvp,
        tc.tile_pool(name="qp", bufs=4, space="PSUM") as qpp,
        tc.tile_pool(name="vp", bufs=2, space="PSUM") as vpp,
    ):
        # resident weights, each loaded write-once
        # wq/wk: [d%128, pair g, d-chunk, (2 heads x 64)]
        wq_sb = wqkp.tile([P, NG, ND, P], BF16, tag="wq_sb")
        wk_sb = wqkp.tile([P, NG, ND, P], BF16, tag="wk_sb")
        nc.sync.dma_start(out=wq_sb, in_=wq_d)
        nc.sync.dma_start(out=wk_sb, in_=wk_d)
        # wv: [d%128, d-chunk, (16 heads x 64)]
        wv_sb = wvp.tile([P, ND, H * E], BF16, tag="wv_sb")
        nc.sync.dma_start(out=wv_sb, in_=wv_d)

        # Q^T / K^T per head pair; bias rank-1 matmul FIRST in each group
        for dst, w_sb, b_sb in ((qT, wq_sb, bq_sb), (kT, wk_sb, bk_sb)):
            for g in range(NG):
                qps = [qpp.tile([P, S // 2], F32, tag="qp", name=f"qp{g}{i}") for i in range(2)]
                for qh in range(2):
                    nc.tensor.matmul(
                        out=qps[qh],
                        lhsT=b_sb[:, g * P : (g + 1) * P],
                        rhs=ones_row[:, qh * 512 : qh * 512 + 512],
                        start=True,
                        stop=False,
                    )
                for c in range(ND):
                    for qh in range(2):  # same lhsT back-to-back
                        nc.tensor.matmul(
                            out=qps[qh],
                            lhsT=w_sb[:, g, c, :],
                            rhs=xT[:, c, qh * 512 : (qh + 1) * 512],
                            start=False,
                            stop=(c == ND - 1),
                        )
                for qh in range(2):
                    nc.vector.tensor_copy(
                        out=dst[:, g, qh * 512 : (qh + 1) * 512], in_=qps[qh]
                    )

        # V = x @ W_V + b_V in two 8-head halves, stored bf16 into vb
        for st in range(NS):
            vps = [vpp.tile([P, 512], F32, tag="vp", name=f"vp{st}{i}") for i in range(2)]
            for hh in range(2):  # same lhsT (ones) back-to-back
                nc.tensor.matmul(
                    out=vps[hh],
                    lhsT=ones_col,
                    rhs=bv_sb[:, hh * 512 : (hh + 1) * 512],
                    start=True,
                    stop=False,
                )
            for c in range(ND):
                for hh in range(2):  # same lhsT (xT chunk) back-to-back
                    nc.tensor.matmul(
                        out=vps[hh],
                        lhsT=xT[:, c, st * P : (st + 1) * P],
                        rhs=wv_sb[:, c, hh * 512 : (hh + 1) * 512],
                        start=False,
                        stop=(c == ND - 1),
                    )
            for hh in range(2):
                nc.vector.tensor_copy(
                    out=vb[:, st, hh * 8 : (hh + 1) * 8, 0:E],
                    in_=vps[hh].rearrange("p (h e) -> p h e", h=8),
                )



def _attention(nc, tc, qT, kT, vb, zT, mb_sb, ones_row, ones_col):
    with (
        tc.tile_pool(name="pt", bufs=2) as ptp,
        tc.tile_pool(name="rcp", bufs=1) as rcp,
        tc.tile_pool(name="sm", bufs=4) as smp,
        tc.tile_pool(name="st", bufs=2, space="PSUM") as stp,
        tc.tile_pool(name="zps", bufs=4, space="PSUM") as zpsp,
    ):
        # write-once strip of softmax denominator reciprocals (bf16)
        rc_all = rcp.tile([1, H, 2, 512], BF16, tag="rc_all")
        for h in range(H):
            g, half = h // 2, h % 2
            pt = ptp.tile([P, NS, S], BF16, tag="pt", name=f"pt{h}")
            for kt in range(NS):
                st_ps = stp.tile([P, S], F32, tag="st", name=f"st{h}{kt}")
                for qh in range(2):
                    nc.tensor.matmul(
                        out=st_ps[:, qh * 512 : (qh + 1) * 512],
                        lhsT=kT[half * E : (half + 1) * E, g,
                                kt * P : (kt + 1) * P],
                        rhs=qT[half * E : (half + 1) * E, g,
                               qh * 512 : (qh + 1) * 512],
                        start=True,
                        stop=True,
                    )
                nc.scalar.activation(
                    out=pt[:, kt, :],
                    in_=st_ps,
                    func=AF.Exp,
                    bias=mb_sb[:, kt : kt + 1],
                    scale=0.125,
                )
            if h == 0:
                nc.tensor.ldweights(vb[:, NS - 1, H - 1, :])
            zps = [zpsp.tile([E + 1, 512], F32, tag="zp", name=f"zp{h}{i}") for i in range(2)]
            for kt in range(NS):
                for qh in range(2):
                    nc.tensor.matmul(
                        out=zps[qh],
                        lhsT=vb[:, kt, h, :],
                        rhs=pt[:, kt, qh * 512 : (qh + 1) * 512],
                        start=(kt == 0),
                        stop=(kt == NS - 1),
                    )
            for qh in range(2):
                zp = zps[qh]
                rc = rc_all[:, h, qh, :]
                with nc.allow_low_precision(reason="bf16 softmax denom"):
                    nc.vector.reciprocal(out=rc, in_=zp[E : E + 1, :])
                bc = smp.tile([E, 512], BF16, tag="bc", name=f"bc{h}{qh}")
                nc.sync.dma_start(
                    out=bc, in_=rc.unsqueeze(1).broadcast_to((1, E, 512))
                )
                nc.vector.tensor_mul(
                    zT[half * E : (half + 1) * E, g, qh * 512 : (qh + 1) * 512],
                    zp[0:E, :],
                    bc,
                )


def _out_proj(nc, tc, zT, wo_sb, bo_sb, ones_col, out_d):
    with (
        tc.tile_pool(name="ob", bufs=1) as obp,
        tc.tile_pool(name="op", bufs=4, space="PSUM") as opp,
    ):
        for st in range(NS):
            ob = obp.tile([P, D], F32, tag=f"ob{st}")  # write-once per s-tile
            ops = [opp.tile([P, 512], F32, tag="op", name=f"op{st}{i}") for i in range(2)]
            for dh in range(2):  # same lhsT (ones) back-to-back
                nc.tensor.matmul(
                    out=ops[dh],
                    lhsT=ones_col,
                    rhs=bo_sb[:, dh * 512 : (dh + 1) * 512],
                    start=True,
                    stop=False,
                )
            for g in range(NG):
                for dh in range(2):  # same lhsT (zT slice) back-to-back
                    nc.tensor.matmul(
                        out=ops[dh],
                        lhsT=zT[:, g, st * P : (st + 1) * P],
                        rhs=wo_sb[:, g, dh * 512 : (dh + 1) * 512],
                        start=False,
                        stop=(g == NG - 1),
                    )
            for dh in range(2):
                nc.vector.tensor_copy(
                    out=ob[:, dh * 512 : (dh + 1) * 512], in_=ops[dh]
                )
            nc.sync.dma_start(out=out_d[st * P : (st + 1) * P, :], in_=ob)


_NC_CACHE = None


def _get_nc():
    global _NC_CACHE
    if _NC_CACHE is None:
        _NC_CACHE = build_program()
    return _NC_CACHE


def _make_in_maps(inputs):
    import ml_dtypes

    bf16 = ml_dtypes.bfloat16
    x = np.asarray(inputs["x"], np.float32)
    mask = np.asarray(inputs["key_attention_mask"])
    wq = np.asarray(inputs["W_Q"], np.float32).astype(bf16)
    wk = np.asarray(inputs["W_K"], np.float32).astype(bf16)
    wv = np.asarray(inputs["W_V"], np.float32).astype(bf16)
    wo = np.asarray(inputs["W_O"], np.float32).astype(bf16)

    def pack_qk(w):  # (H, D, E) -> [p, g, c, (h2 e)]
        return np.ascontiguousarray(
            w.reshape(NG, 2, ND, P, E).transpose(3, 0, 2, 1, 4).reshape(P, NG, ND, P)
        )

    cpk = np.concatenate([
        np.asarray(inputs["b_Q"], np.float32).reshape(-1),
        np.asarray(inputs["b_K"], np.float32).reshape(-1),
        np.asarray(inputs["b_V"], np.float32).reshape(-1),
        np.asarray(inputs["b_O"], np.float32).reshape(-1),
    ]).reshape(1, 4 * 1024)
    shared = {
        "wq": pack_qk(wq),
        "wk": pack_qk(wk),
        # (H, D, E) -> [p, c, (h e)]
        "wv": np.ascontiguousarray(
            wv.reshape(H, ND, P, E).transpose(2, 1, 0, 3).reshape(P, ND, H * E)
        ),
        # (H, E, D) -> [(h2 e), g, d]
        "wo": np.ascontiguousarray(
            wo.reshape(NG, 2, E, D).transpose(1, 2, 0, 3).reshape(P, NG, D)
        ),
        "cpk": cpk.astype(bf16),
    }
    in_maps = []
    for b in range(B):
        m = dict(shared)
        xt = x[b].T.astype(bf16)  # (D, S) -> [p, c, s]
        m["xt"] = np.ascontiguousarray(
            xt.reshape(ND, P, S).transpose(1, 0, 2)
        )
        mb = ((mask[b] != 0).astype(np.float32) - 1.0) * MASK_NEG
        m["mb"] = np.ascontiguousarray(mb.reshape(NS, P).T)
        in_maps.append(m)
    return in_maps


def run(inputs, trace=False):
    nc = _get_nc()
    res = run_bass_kernel_spmd(nc, _make_in_maps(inputs), list(range(B)),
                               trace=trace)
    out = np.stack([res.results[b]["out"] for b in range(B)], axis=0)
    return out, res


def kernel(**inputs) -> np.ndarray:
    out, _ = run(inputs, trace=False)
    return out



# revision 4
# speedup vs baseline: 1.5233x; 1.5233x over previous
"""Multi-head attention forward on 8 Trainium2 NeuronCores.

Problem: batch=8, seq=1024, d_model=1024, n_heads=16, d_head=64, fp32 ref.

Sharding: data-parallel over batch - core b computes batch element b end to
end (weights replicated, no collectives).

v2: software-pipelined head-pair schedule. The kernel is one long loop over
head PAIRS where the PE never idles >1us (keeps the HAM clock gate at
K=8/8 = 2.4 GHz; the v1 phase-split kernel spent ~200us re-throttled at
1.2 GHz because the PE sat idle behind softmax exp):

  prologue: Q/K projection for pair 0 + V projection heads 0-7
  slot g:   scores(g) row-tile-paired + exp(g) + PV(g)
            interleaved with QK-proj(g+1) and V-proj heads 8-15
  tail:     output projection per s-tile

Per-engine layout tricks:
  - scores^T = K @ Q^T per head has K(contraction)=64: the two heads of a
    pair live on partition halves 0-63 / 64-127, so their score matmuls
    auto-derive PE row-tile positions (0,0)/(64,0) and run CONCURRENTLY in
    the systolic array (2x score throughput).
  - softmax: exp on ScalarE with the 1/8 scale and key-mask folded in as
    activation scale/bias; denominators pop out of the PV matmul via a
    ones-column appended to V (psum row 64).
  - denominator reciprocals are done on a [128,16] tile (one per pair)
    instead of [1,512] strips - the v1 kernel burned 127us of DVE on
    1-partition reciprocals.
  - b_Q/b_K are added by DVE tensor_scalar (per-partition scalar AP) during
    the psum->sbuf copy; b_V is folded into b_O on the host
    (out = (Z/den + b_V) W_O + b_O = Z/den W_O + (b_V W_O + b_O)).

Everything is bf16 into the PE with fp32 PSUM accumulation.

This toolchain's walrus encodes at most ONE sync wait per instruction;
_split_multi_waits hoists excess waits onto same-engine EventSemaphore
instructions.
"""

from contextlib import ExitStack

import numpy as np

import concourse.bass as bass
import concourse.tile as tile
from concourse import mybir
from concourse.bass_utils import run_bass_kernel_spmd

S = 1024  # seq
D = 1024  # d_model
H = 16  # heads
E = 64  # d_head
B = 8  # batch == n_cores
P = 128  # partitions
NS = S // P  # 8 s-tiles
ND = D // P  # 8 d-chunks
NG = H // 2  # 8 head pairs

F32 = mybir.dt.float32
BF16 = mybir.dt.bfloat16
AF = mybir.ActivationFunctionType
ADD = mybir.AluOpType.add

MASK_NEG = 60.0  # exp(x - 60) ~ 9e-27: masked keys vanish without inf/nan


def build_program(split_waits=True):
    nc = bass.Bass("TRN2", target_bir_lowering=False, debug=False)

    xt_d = nc.dram_tensor("xt", [P, ND, S], BF16, kind="ExternalInput").ap()
    wq_d = nc.dram_tensor("wq", [P, NG, ND, P], BF16, kind="ExternalInput").ap()
    wk_d = nc.dram_tensor("wk", [P, NG, ND, P], BF16, kind="ExternalInput").ap()
    wv_d = nc.dram_tensor("wv", [P, ND, H * E], BF16, kind="ExternalInput").ap()
    wo_d = nc.dram_tensor("wo", [P, NG, D], BF16, kind="ExternalInput").ap()
    # b_Q / b_K as per-pair partition columns: [:, 0:8]=b_Q, [:, 8:16]=b_K
    bqk_d = nc.dram_tensor("bqk", [P, 2 * NG], F32, kind="ExternalInput").ap()
    bo_d = nc.dram_tensor("bo", [1, D], BF16, kind="ExternalInput").ap()
    mb_d = nc.dram_tensor("mb", [P, NS], F32, kind="ExternalInput").ap()
    out_d = nc.dram_tensor("out", [S, D], F32, kind="ExternalOutput").ap()

    with tile.TileContext(nc) as tc, ExitStack() as ctx:
        g1 = ctx.enter_context(tc.tile_pool(name="g1", bufs=1))

        ones_col = g1.tile([1, P], BF16, tag="ones_col")
        nc.vector.memset(ones_col, 1.0)
        mb_sb = g1.tile([P, NS], F32, tag="mb")
        nc.sync.dma_start(out=mb_sb, in_=mb_d)
        bqk = g1.tile([P, 2 * NG], F32, tag="bqk")
        nc.sync.dma_start(out=bqk, in_=bqk_d)
        bo_sb = g1.tile([1, D], BF16, tag="bo")
        nc.sync.dma_start(out=bo_sb, in_=bo_d)

        # weights / activations, streamed in fine slices so the prologue's
        # first matmuls start ~2us in
        wq_sb = g1.tile([P, NG, ND, P], BF16, tag="wq_sb")
        wk_sb = g1.tile([P, NG, ND, P], BF16, tag="wk_sb")
        xT = g1.tile([P, ND, S], BF16, tag="xT")
        wv_sb = g1.tile([P, ND, H * E], BF16, tag="wv_sb")
        wo_sb = g1.tile([P, NG, D], BF16, tag="wo_sb")
        nc.sync.dma_start(out=wq_sb[:, 0], in_=wq_d[:, 0])
        nc.sync.dma_start(out=wk_sb[:, 0], in_=wk_d[:, 0])
        for c in range(ND):
            nc.sync.dma_start(out=xT[:, c], in_=xt_d[:, c])
        for g in range(1, NG):
            nc.sync.dma_start(out=wq_sb[:, g], in_=wq_d[:, g])
            nc.sync.dma_start(out=wk_sb[:, g], in_=wk_d[:, g])
        for c in range(ND):
            nc.sync.dma_start(out=wv_sb[:, c], in_=wv_d[:, c])
        nc.sync.dma_start(out=wo_sb, in_=wo_d)

        # persistent activations
        qT = g1.tile([P, NG, S], BF16, tag="qT")
        kT = g1.tile([P, NG, S], BF16, tag="kT")
        vb = g1.tile([P, NS, H, E + 1], BF16, tag="vb")
        nc.vector.memset(vb, 1.0)  # pre-sets the softmax-sum ones columns
        zT = g1.tile([P, NG, S], BF16, tag="zT")

        # observer instructions: absorb one new semaphore tick each so later
        # consumers of shared tensors carry at most one wait themselves.
        nc.tensor.ldweights(ones_col)  # DVE tick (memsets)
        nc.tensor.ldweights(xT[:, 0, 0:8])  # xT chunk-0 DMA lane
        nc.tensor.ldweights(wo_sb[:, 0, 0:8])  # wo DMA lane
        nc.tensor.ldweights(bo_sb[:, 0:P])  # bo DMA lane
        act_scrap = g1.tile([P, 1], F32, tag="act_scrap")
        nc.scalar.activation(  # mb DMA lane, observed by ScalarE
            out=act_scrap, in_=mb_sb[:, 0:1], func=AF.Copy
        )

        # ---- prologue: QK proj pair 0 + V heads 0-7, double-buffered psum
        def qk_group(pool, dst, w_sb, g, qh, bcol):
            qp = pool.tile([P, 512], F32, tag="qp")
            for c in range(ND):
                nc.tensor.matmul(
                    out=qp,
                    lhsT=w_sb[:, g, c, :],
                    rhs=xT[:, c, qh * 512 : (qh + 1) * 512],
                    start=(c == 0),
                    stop=(c == ND - 1),
                )
            with nc.allow_low_precision(reason="bf16 q/k with fused bias"):
                nc.vector.tensor_scalar(
                    out=dst[:, g, qh * 512 : (qh + 1) * 512],
                    in0=qp,
                    scalar1=bcol,
                    scalar2=None,
                    op0=ADD,
                )

        def v_group(pool, st, hh):
            vp = pool.tile([P, 512], F32, tag="qp")
            for c in range(ND):
                nc.tensor.matmul(
                    out=vp,
                    lhsT=xT[:, c, st * P : (st + 1) * P],
                    rhs=wv_sb[:, c, hh * 512 : (hh + 1) * 512],
                    start=(c == 0),
                    stop=(c == ND - 1),
                )
            nc.vector.tensor_copy(
                out=vb[:, st, hh * 8 : (hh + 1) * 8, 0:E],
                in_=vp.rearrange("p (h e) -> p h e", h=8),
            )

        with tc.tile_pool(name="qpro", bufs=2, space="PSUM") as qpro:
            for qh in range(2):
                qk_group(qpro, qT, wq_sb, 0, qh, bqk[:, 0:1])
            for qh in range(2):
                qk_group(qpro, kT, wk_sb, 0, qh, bqk[:, NG : NG + 1])
            for st in range(NS):
                v_group(qpro, st, 0)

        # ---- pipelined head-pair slots
        with (
            tc.tile_pool(name="qpp", bufs=1, space="PSUM") as qpp,
            tc.tile_pool(name="stp", bufs=2, space="PSUM") as stp,
            tc.tile_pool(name="ztp", bufs=3, space="PSUM") as ztp,
            tc.tile_pool(name="ptp", bufs=18) as ptp,
            tc.tile_pool(name="zsbp", bufs=2) as zsbp,
            tc.tile_pool(name="denp", bufs=2) as denp,
            tc.tile_pool(name="rcp", bufs=2) as rcp,
            tc.tile_pool(name="bcp", bufs=4) as bcp,
        ):
            for g in range(NG):
                hA, hB = 2 * g, 2 * g + 1
                # filler matmul groups to keep the PE busy while ScalarE exps
                work = []
                if g < NG - 1:
                    for qh in range(2):
                        work.append((qk_group, qpp, qT, wq_sb, g + 1, qh,
                                     bqk[:, g + 1 : g + 2]))
                    for qh in range(2):
                        work.append((qk_group, qpp, kT, wk_sb, g + 1, qh,
                                     bqk[:, NG + g + 1 : NG + g + 2]))
                if g < 3:
                    for st in range(3 * g, min(3 * g + 3, NS)):
                        work.append((v_group, qpp, st, 1))
                work.reverse()

                pts = []
                ztA0 = ztB0 = None
                for kt in range(NS):
                    stA = stp.tile([P, S], F32, tag="st", name=f"stA{g}{kt}")
                    stB = stp.tile([P, S], F32, tag="st", name=f"stB{g}{kt}")
                    for qh in range(2):
                        # heads of a pair sit on partition halves -> PE
                        # row-tiles (0,0)/(64,0), concurrent in the array
                        nc.tensor.matmul(
                            out=stA[:, qh * 512 : (qh + 1) * 512],
                            lhsT=kT[0:E, g, kt * P : (kt + 1) * P],
                            rhs=qT[0:E, g, qh * 512 : (qh + 1) * 512],
                            start=True,
                            stop=True,
                        )
                        nc.tensor.matmul(
                            out=stB[:, qh * 512 : (qh + 1) * 512],
                            lhsT=kT[E:P, g, kt * P : (kt + 1) * P],
                            rhs=qT[E:P, g, qh * 512 : (qh + 1) * 512],
                            start=True,
                            stop=True,
                        )
                    ptA = ptp.tile([P, S], BF16, tag="pt", name=f"ptA{g}{kt}")
                    ptB = ptp.tile([P, S], BF16, tag="pt", name=f"ptB{g}{kt}")
                    nc.scalar.activation(
                        out=ptA, in_=stA, func=AF.Exp,
                        bias=mb_sb[:, kt : kt + 1], scale=0.125,
                    )
                    nc.scalar.activation(
                        out=ptB, in_=stB, func=AF.Exp,
                        bias=mb_sb[:, kt : kt + 1], scale=0.125,
                    )
                    pts.append((ptA, ptB))
                    if kt == 0:
                        ztA0 = ztp.tile([E + 1, 512], F32, tag="zt", name=f"ztA0{g}")
                        ztB0 = ztp.tile([E + 1, 512], F32, tag="zt", name=f"ztB0{g}")
                    nc.tensor.matmul(
                        out=ztA0, lhsT=vb[:, kt, hA, :], rhs=ptA[:, 0:512],
                        start=(kt == 0), stop=(kt == NS - 1),
                    )
                    nc.tensor.matmul(
                        out=ztB0, lhsT=vb[:, kt, hB, :], rhs=ptB[:, 0:512],
                        start=(kt == 0), stop=(kt == NS - 1),
                    )
                    if work:
                        fn, pool, *args = work.pop()
                        fn(pool, *args)
                # drain leftover filler groups (none in steady state)
                while work:
                    fn, pool, *args = work.pop()
                    fn(pool, *args)

                # z rows 0-63 + denominator row 64, moved psum->sbuf early so
                # the psum banks recycle without waiting on the normalize
                zsb = zsbp.tile([E + 1, 4, 512], BF16, tag="zsb", name=f"zsb{g}")
                with nc.allow_low_precision(reason="bf16 z/denominator"):
                    nc.vector.tensor_copy(out=zsb[:, 0, :], in_=ztA0)
                    nc.vector.tensor_copy(out=zsb[:, 1, :], in_=ztB0)
                ztA1 = ztp.tile([E + 1, 512], F32, tag="zt", name=f"ztA1{g}")
                for kt in range(NS):
                    nc.tensor.matmul(
                        out=ztA1, lhsT=vb[:, kt, hA, :], rhs=pts[kt][0][:, 512:1024],
                        start=(kt == 0), stop=(kt == NS - 1),
                    )
                ztB1 = ztp.tile([E + 1, 512], F32, tag="zt", name=f"ztB1{g}")
                for kt in range(NS):
                    nc.tensor.matmul(
                        out=ztB1, lhsT=vb[:, kt, hB, :], rhs=pts[kt][1][:, 512:1024],
                        start=(kt == 0), stop=(kt == NS - 1),
                    )
                with nc.allow_low_precision(reason="bf16 z/denominator"):
                    nc.vector.tensor_copy(out=zsb[:, 2, :], in_=ztA1)
                    nc.vector.tensor_copy(out=zsb[:, 3, :], in_=ztB1)

                # normalize: one well-shaped reciprocal per pair
                den_sq = denp.tile([P, 16], BF16, tag="den", name=f"den{g}")
                nc.sync.dma_start(out=den_sq, in_=zsb[E : E + 1, :, :])
                rc_sq = rcp.tile([P, 16], BF16, tag="rc", name=f"rc{g}")
                with nc.allow_low_precision(reason="bf16 softmax denom"):
                    nc.vector.reciprocal(out=rc_sq, in_=den_sq)
                rc_strip = rcp.tile([1, 4, 512], BF16, tag="rcs", name=f"rcs{g}")
                nc.sync.dma_start(out=rc_strip, in_=rc_sq)
                for i, (half, qh) in enumerate(((0, 0), (1, 0), (0, 1), (1, 1))):
                    bc = bcp.tile([E, 512], BF16, tag="bc", name=f"bc{g}{i}")
                    nc.sync.dma_start(
                        out=bc,
                        in_=rc_strip[:, i, :].unsqueeze(1).broadcast_to((1, E, 512)),
                    )
                    nc.vector.tensor_mul(
                        zT[half * E : (half + 1) * E, g, qh * 512 : (qh + 1) * 512],
                        zsb[0:E, i, :],
                        bc,
                    )

        # ---- tail: output projection
        with (
            tc.tile_pool(name="obp", bufs=2) as obp,
            tc.tile_pool(name="opp", bufs=4, space="PSUM") as opp,
        ):
            for st in range(NS):
                ob = obp.tile([P, D], F32, tag="ob", name=f"ob{st}")
                ops = [opp.tile([P, 512], F32, tag="op", name=f"op{st}{i}")
                       for i in range(2)]
                for dh in range(2):  # bias first: WAR wait rides on it
                    nc.tensor.matmul(
                        out=ops[dh],
                        lhsT=ones_col,
                        rhs=bo_sb[:, dh * 512 : (dh + 1) * 512],
                        start=True,
                        stop=False,
                    )
                for g in range(NG):
                    for dh in range(2):
                        nc.tensor.matmul(
                            out=ops[dh],
                            lhsT=zT[:, g, st * P : (st + 1) * P],
                            rhs=wo_sb[:, g, dh * 512 : (dh + 1) * 512],
                            start=False,
                            stop=(g == NG - 1),
                        )
                for dh in range(2):
                    nc.vector.tensor_copy(
                        out=ob[:, dh * 512 : (dh + 1) * 512], in_=ops[dh]
                    )
                nc.sync.dma_start(out=out_d[st * P : (st + 1) * P, :], in_=ob)

    if split_waits:
        _split_multi_waits(nc)
    return nc


def _split_multi_waits(nc):
    """This walrus build encodes at most ONE sync wait per instruction.
    Tile emits more. Hoist excess waits onto same-engine EventSemaphore
    instructions inserted immediately before the offender - engines and
    DGE sequencers execute their streams in order, so this preserves
    semantics exactly."""
    n = 0
    for fn in nc.m.functions:
        for bb in fn.blocks:
            out = []
            for inst in bb.instructions:
                si = getattr(inst, "sync_info", None)
                waits = list(si.on_wait) if si is not None and si.on_wait else []
                if len(waits) > 1:
                    for w in waits[:-1]:
                        n += 1
                        out.append(
                            mybir.InstEventSemaphore(
                                name=f"evw-{n}",
                                engine=inst.engine,
                                sync_info=mybir.SyncInfo(
                                    on_wait=[w], on_update=[]
                                ),
                            )
                        )
                    si.on_wait = [waits[-1]]
                out.append(inst)
            bb.instructions[:] = out


_NC_CACHE = None


def _get_nc():
    global _NC_CACHE
    if _NC_CACHE is None:
        _NC_CACHE = build_program()
    return _NC_CACHE


def _make_in_maps(inputs):
    import ml_dtypes

    bf16 = ml_dtypes.bfloat16
    x = np.asarray(inputs["x"], np.float32)
    mask = np.asarray(inputs["key_attention_mask"])
    wq = np.asarray(inputs["W_Q"], np.float32).astype(bf16)
    wk = np.asarray(inputs["W_K"], np.float32).astype(bf16)
    wv = np.asarray(inputs["W_V"], np.float32).astype(bf16)
    wo = np.asarray(inputs["W_O"], np.float32).astype(bf16)

    def pack_qk(w):  # (H, D, E) -> [p, g, c, (h2 e)]
        return np.ascontiguousarray(
            w.reshape(NG, 2, ND, P, E).transpose(3, 0, 2, 1, 4).reshape(P, NG, ND, P)
        )

    def pack_bcol(b):  # (H, E) -> [(h2 e), g]
        return b.reshape(NG, 2, E).transpose(1, 2, 0).reshape(P, NG)

    bqk = np.concatenate(
        [
            pack_bcol(np.asarray(inputs["b_Q"], np.float32)),
            pack_bcol(np.asarray(inputs["b_K"], np.float32)),
        ],
        axis=1,
    )
    # fold b_V into b_O: out = (Z/den + b_V) W_O + b_O
    bo = np.asarray(inputs["b_O"], np.float64) + np.einsum(
        "he,hed->d",
        np.asarray(inputs["b_V"], np.float64),
        np.asarray(inputs["W_O"], np.float64),
    )
    shared = {
        "wq": pack_qk(wq),
        "wk": pack_qk(wk),
        # (H, D, E) -> [p, c, (h e)]
        "wv": np.ascontiguousarray(
            wv.reshape(H, ND, P, E).transpose(2, 1, 0, 3).reshape(P, ND, H * E)
        ),
        # (H, E, D) -> [(h2 e), g, d]
        "wo": np.ascontiguousarray(
            wo.reshape(NG, 2, E, D).transpose(1, 2, 0, 3).reshape(P, NG, D)
        ),
        "bqk": np.ascontiguousarray(bqk),
        "bo": bo.astype(np.float32).astype(bf16).reshape(1, D),
    }
    in_maps = []
    for b in range(B):
        m = dict(shared)
        xt = x[b].T.astype(bf16)  # (D, S) -> [p, c, s]
        m["xt"] = np.ascontiguousarray(
            xt.reshape(ND, P, S).transpose(1, 0, 2)
        )
        mb = ((mask[b] != 0).astype(np.float32) - 1.0) * MASK_NEG
        m["mb"] = np.ascontiguousarray(mb.reshape(NS, P).T)
        in_maps.append(m)
    return in_maps


def run(inputs, trace=False):
    nc = _get_nc()
    res = run_bass_kernel_spmd(nc, _make_in_maps(inputs), list(range(B)),
                               trace=trace)
    out = np.stack([res.results[b]["out"] for b in range(B)], axis=0)
    return out, res


def kernel(**inputs) -> np.ndarray:
    out, _ = run(inputs, trace=False)
    return out
